# revision 47
# baseline (speedup 1.0000x reference)
"""Trainium2 Bass kernel for GQA attention with RoPE (tensor-parallel over heads).

Reference computation (per problem spec):
  x:[1,2048,4096], wq:[4096,4096], wk/wv:[4096,1024], wo:[4096,4096], f32
  q/k/v proj -> RoPE(q,k) -> causal GQA softmax attention -> o_proj

Sharding: 8 cores, tensor-parallel over heads. Core c gets 4 query heads
(wq cols [c*512:(c+1)*512]) and 1 KV head (wk/wv cols [c*128:(c+1)*128]),
plus wo rows [c*512:(c+1)*512]. Each core computes a full [2048,4096]
partial o_proj output; the host sums the 8 partials (the all-reduce).
The host dispatch layer hands the device x pre-transposed ([D,S]).

Matmul operands are fp16 (fast-weight-load hides LDWEIGHTS under the
512-col stream; steady cadence = 216 ns/MM = N/2.4GHz + dispatch).
All accumulation is fp32 in PSUM; softmax statistics fp32.

Trace-driven optimizations over the 477us baseline (now ~388us):
  - softmax denominators: the per-chunk ones-matmul (160 extra 512-col
    PE streams ~= 31us) is replaced by a DVE add folding each exp tile
    into a per-head [128,512] fp16 partial-sum, reduced by ONE
    ones-matmul per (qi,h). Frees a PSUM bank (st*2 + oacc*2 + sacc +
    opj*3 = 8).
  - software pipelining: the PV matmul of chunk c-1 is emitted after
    the score matmul of chunk c, so its exp/mask dependency is long
    satisfied when PE reaches it (the PE 4-deep wait queue covers the
    rest).
  - attention runs at single-chunk granularity ([128,512] score tiles,
    one exp ACT per chunk); oacc double-buffering kills the head-
    boundary stall where PV-start(h+1) waited on the norm mul(h).
  - diagonal score/PV matmuls narrowed to the causally-valid column
    range; the in-chunk triangle is one [128,128] mask multiply.
  - o_proj chunks are interleaved into later q-tiles' chunk loops at a
    rate proportional to the backlog (early q-tiles are ACT-bound, so
    injected PE work rides free); leftovers drain at q-tile ends.
  - dependency granularity: wq/qT/outhT are per-head tiles and vnat
    per-strip (tile-granular tracking -- a consumer only waits on the
    producer it actually reads, not on the tile's latest writer).
  - warm-up: dummy gpsimd partition_broadcast loads the Q7 library and
    a dummy exp loads the ScalarE activation table during phase-1
    startup (each otherwise costs 1.3-7us at the phase boundary).
  - startup DMA schedule in consumption order (k/v sweeps first: their
    1MB weights land before the 4MB wq; consts later); xt prefetched
    one strip ahead; output DMAs per [128,2048] half-strip, per-chunk
    for the final strip so the last transfer is 128KB.
"""
import numpy as np

import concourse.bass as bass
import concourse.bacc as bacc
import concourse.tile as tile
import concourse.mybir as mybir
from concourse import bass_utils

F32 = mybir.dt.float32
F16 = mybir.dt.float16
AF = mybir.ActivationFunctionType

# model dims (hardcoded per problem spec nn_Attention_52020643889298)
S = 2048
D = 4096
H = 32
KV = 8
HD = 128
THETA = 10000.0
NCORES = 8
HQ = H // NCORES            # 4 query heads per core
NQ = HQ * HD                # 512 wq cols per core
NKV = (KV // NCORES) * HD   # 128 wk/wv cols per core

# tiling
SSTRIP = 512                # phase-1 s-strip
NSTRIPS = S // SSTRIP       # 4
NSUB = SSTRIP // 128        # 4
DCH = D // 128              # 32 contraction chunks
QTILE = 512                 # attention q-tile
NQT = S // QTILE            # 4
RD = QTILE // 128           # 4 key chunks per q-tile on the diagonal
NPCH = S // 128             # 16 key chunks

EXP_BIAS = -10.0            # exp(s-10): keeps exp in fp16 range; cancels
                            # in the softmax normalization


def _rope_tables():
    inv = 1.0 / (THETA ** (np.arange(0, HD, 2, dtype=np.float64) / HD))
    pos = np.arange(S, dtype=np.float64)
    freqs = pos[:, None] * inv[None, :]          # [S, 64]
    emb = np.concatenate([freqs, freqs], axis=1)  # [S, HD]
    cosT = np.cos(emb).T.astype(np.float16).copy()  # [HD, S]
    sinT = np.sin(emb).T.astype(np.float16).copy()
    return cosT, sinT


def _mask_tri():
    # mask[p, j] = 1 iff j >= p: the in-chunk causal triangle. Every
    # diagonal [128-key x 128-query] block uses this same mask after the
    # score tile is narrowed to its first causally-live 128 columns.
    p = np.arange(128)[:, None]
    j = np.arange(128)[None, :]
    return (j >= p).astype(np.float16)


def build():
    nc = bacc.Bacc("TRN2", target_bir_lowering=False, debug=False,
                   enable_asserts=False, num_devices=NCORES)
    xt_d = nc.dram_tensor("xt", [D, S], F16, kind="ExternalInput").ap()
    wq_d = nc.dram_tensor("wq", [D, NQ], F16, kind="ExternalInput").ap()
    wk_d = nc.dram_tensor("wk", [D, NKV], F16, kind="ExternalInput").ap()
    wv_d = nc.dram_tensor("wv", [D, NKV], F16, kind="ExternalInput").ap()
    wo_d = nc.dram_tensor("wo", [NQ, D], F16, kind="ExternalInput").ap()
    out_d = nc.dram_tensor("out", [S, D], F16, kind="ExternalOutput").ap()

    cosT, sinT = _rope_tables()
    ident_d = nc.inline_tensor(np.eye(128, dtype=np.float16), "ident").ap()
    cos_d = nc.inline_tensor(cosT, "cosT").ap()
    sin_d = nc.inline_tensor(sinT, "sinT").ap()
    mask_d = nc.inline_tensor(_mask_tri(), "masktri").ap()
    ones_d = nc.inline_tensor(np.ones((128, 1), np.float16), "onescol").ap()
    ebias_d = nc.inline_tensor(
        np.full((128, 1), EXP_BIAS, np.float32), "ebias").ap()

    with tile.TileContext(nc) as tc:
        _body(nc, tc, xt_d, wq_d, wk_d, wv_d, wo_d, out_d,
              ident_d, cos_d, sin_d, mask_d, ones_d, ebias_d)
    nc.compile()
    return nc


def _body(nc, tc, xt_d, wq_d, wk_d, wv_d, wo_d, out_d,
          ident_d, cos_d, sin_d, mask_d, ones_d, ebias_d):
    wqr = wq_d.rearrange("(c p) n -> p c n", p=128)
    wkr = wk_d.rearrange("(c p) n -> p c n", p=128)
    wvr = wv_d.rearrange("(c p) n -> p c n", p=128)
    xtr = xt_d.rearrange("(c p) s -> p c s", p=128)  # [128, DCH, S]

    with tc.tile_pool(name="const", bufs=1) as const_pool, \
         tc.tile_pool(name="persist", bufs=1) as persist:

        # persistent activations. qT is one tile per head and vnat one
        # tile per strip: dependency tracking is tile-granular, so the
        # first q-tile's score/PV matmuls only wait on the strips they
        # actually read, not on the last strip's rope/transpose chains.
        qT_sb = [persist.tile([128, S], F16, name=f"qT{g}")
                 for g in range(HQ)]               # [hd, s] x HQ
        kT_sb = persist.tile([128, S], F16)        # [hd, s]
        vnat_sb = [persist.tile([128, NSUB, HD], F16, name=f"vnat{si}")
                   for si in range(NSTRIPS)]       # [s%128, sub, hd]

        # ---------------- phase 1: QKV projection + RoPE ----------------
        wo_pool_cm = tc.tile_pool(name="wo2", bufs=1)
        outh_pool_cm = tc.tile_pool(name="outh", bufs=1)
        wo_pool = wo_pool_cm.__enter__()
        outh_pool = outh_pool_cm.__enter__()
        wo_sb = wo_pool.tile([128, HQ, D], F16)
        # one tile per head: tile-granular dependency tracking means an
        # o_proj matmul only waits on ITS head's latest norm mul, not on
        # whatever head was normalized most recently.
        outhT_sb = [outh_pool.tile([128, S], F16, name=f"outh{h}")
                    for h in range(HQ)]  # [hd, s] x HQ
        with tc.tile_pool(name="rope_c", bufs=1) as rope_c, \
             tc.tile_pool(name="w1", bufs=1) as w1, \
             tc.tile_pool(name="xt", bufs=12) as xt_pool, \
             tc.tile_pool(name="p1tmp", bufs=2) as p1tmp, \
             tc.tile_pool(name="tp_ps", bufs=2, space="PSUM") as tp_ps, \
             tc.tile_pool(name="acc_ps", bufs=1, space="PSUM") as acc_ps:

            # wq as one tile per head block: a q-sweep then only waits on
            # its own 1MB DMA, not on all 4MB of wq (tile-granular deps)
            wq_sb = [w1.tile([128, DCH, 128], F16, name=f"wq{g}")
                     for g in range(HQ)]
            wk_sb = w1.tile([128, DCH, NKV], F16)
            wv_sb = w1.tile([128, DCH, NKV], F16)

            XG = 4  # d-chunks per xt DMA (512KB per transfer)

            def load_xt(si, j):
                t = xt_pool.tile([128, XG, SSTRIP], F16, tag="xt",
                                 name=f"xt{si}_{j}")
                nc.sync.dma_start(
                    t[:], xtr[:, j * XG:(j + 1) * XG,
                              si * SSTRIP:(si + 1) * SSTRIP])
                return t

            # Startup DMA schedule, in consumption order: the k-sweep
            # (first sweep, DMA-paced) reads xt chunk groups in j order
            # plus the 1MB wk; wv before the xt tail so the v-sweep never
            # waits; the 4MB wq halves and DVE-only consts ride behind.
            xts = {}
            # issue the startup-critical DMAs from three different
            # engines in parallel: each dma_start costs ~650ns of issue
            # time on its engine, so fanning out gets all of the first
            # sweep's transfers in flight several us sooner.
            xts[(0, 0)] = load_xt(0, 0)
            nc.scalar.dma_start(wk_sb[:], wkr[:])
            xts[(0, 1)] = load_xt(0, 1)
            xts[(0, 2)] = load_xt(0, 2)
            xts[(0, 3)] = load_xt(0, 3)
            nc.scalar.dma_start(wv_sb[:], wvr[:])
            for j in range(4, 8):
                t = xt_pool.tile([128, XG, SSTRIP], F16, tag="xt",
                                 name=f"xt0_{j}")
                nc.scalar.dma_start(
                    t[:], xtr[:, j * XG:(j + 1) * XG, 0:SSTRIP])
                xts[(0, j)] = t

            ebias = const_pool.tile([128, 1], F32)
            nc.sync.dma_start(ebias[:], ebias_d[:])
            ones_col = const_pool.tile([128, 1], F16)
            nc.sync.dma_start(ones_col[:], ones_d[:])
            ident = const_pool.tile([128, 128], F16)
            nc.sync.dma_start(ident[:], ident_d[:])

            # Preload the gpsimd library (partition_broadcast lives in a
            # Q7 library that otherwise lazy-loads at first use -- ~7us of
            # dead time right at the attention phase boundary). This dummy
            # broadcast hides the load under the strip-0 sweeps. Same idea
            # for ScalarE's Exp activation table (~1.3us ACT_TABLE_LOAD
            # that would otherwise land on the first real exp).
            dummy_bc = const_pool.tile([128, 1], F32)
            nc.gpsimd.partition_broadcast(dummy_bc[:], ebias[0:1, :],
                                          channels=128)
            dummy_exp = const_pool.tile([128, 1], F16)
            nc.scalar.activation(dummy_exp[:], ebias[:], AF.Exp,
                                 bias=ebias[:])

            for g in range(HQ):
                nc.sync.dma_start(wq_sb[g][:],
                                  wqr[:, :, g * 128:(g + 1) * 128])
            cos_sb = rope_c.tile([128, S], F16)
            nc.sync.dma_start(cos_sb[:], cos_d[:])
            sin_sb = rope_c.tile([128, S], F16)
            nc.sync.dma_start(sin_sb[:], sin_d[:])
            mask_sb = const_pool.tile([128, 128], F16)
            nc.sync.dma_start(mask_sb[:], mask_d[:])

            def rope_store(src_ps, dst_ap, sslice):
                # dst = src*cos + rot(src)*sin, rot = [-src[64:], src[:64]].
                # SBUF+SBUF DVE operands must share their base partition, so
                # materialize the half-rotated src from PSUM first, then all
                # remaining ops are partition-aligned fp16 SBUF math.
                qrot = p1tmp.tile([128, SSTRIP], F16, tag="rope_qr",
                                  name="rope_qr")
                nc.vector.tensor_copy(qrot[0:64, :], src_ps[64:128, :])
                nc.vector.tensor_copy(qrot[64:128, :], src_ps[0:64, :])
                qcos = p1tmp.tile([128, SSTRIP], F16, tag="rope_qc",
                                  name="rope_qc")
                nc.vector.tensor_mul(qcos[:], src_ps[:], cos_sb[:, sslice])
                nc.vector.tensor_mul(qrot[:], qrot[:], sin_sb[:, sslice])
                nc.vector.tensor_sub(dst_ap[0:64, :], qcos[0:64, :],
                                     qrot[0:64, :])
                nc.vector.tensor_add(dst_ap[64:128, :], qcos[64:128, :],
                                     qrot[64:128, :])

            for si in range(NSTRIPS):
                s0 = si * SSTRIP
                sslice = slice(s0, s0 + SSTRIP)
                if si > 0:
                    # rest of this strip's x columns (j0-j3 were
                    # prefetched during the previous strip)
                    for j in range(4, DCH // XG):
                        xts[(si, j)] = load_xt(si, j)

                kacc = acc_ps.tile([128, SSTRIP], F32, tag="kacc")
                vacc = acc_ps.tile([128, SSTRIP], F32, tag="vacc")
                qacc = [acc_ps.tile([128, SSTRIP], F32, tag=f"qacc{g}",
                                    name=f"qacc{g}")
                        for g in range(HQ)]

                xtiles = [xts.pop((si, j)) for j in range(DCH // XG)]

                def sweep(acc, wsl):
                    for j in range(DCH // XG):
                        for jj in range(XG):
                            dc = j * XG + jj
                            nc.tensor.matmul(acc[:], wsl(dc),
                                             xtiles[j][:, jj, :],
                                             start=(dc == 0),
                                             stop=(dc == DCH - 1))

                def sweep_k():
                    sweep(kacc, lambda dc: wk_sb[:, dc, :])
                    rope_store(kacc, kT_sb[:, sslice], sslice)

                def sweep_v():
                    # vstg (DVE) is emitted right after the sweep; the PE
                    # transposes are emitted later (after the next sweep)
                    # so they never queue behind a DVE chain in flight.
                    sweep(vacc, lambda dc: wv_sb[:, dc, :])
                    vstg = p1tmp.tile([128, SSTRIP], F16, tag="vstg")
                    nc.vector.tensor_copy(vstg[:], vacc[:])
                    return vstg

                def transposes(vstg):
                    # all 4 transposes land in one PSUM tile, drained by a
                    # single copy (vnat's strip slice is contiguous) -- no
                    # PE<->DVE ping-pong at strip boundaries
                    tp = tp_ps.tile([128, SSTRIP], F16, tag="tp")
                    for ss in range(NSUB):
                        nc.tensor.transpose(
                            tp[:, ss * 128:(ss + 1) * 128],
                            vstg[:, ss * 128:(ss + 1) * 128], ident[:])
                    nc.vector.tensor_copy(vnat_sb[si][:], tp[:])

                def sweep_q(g):
                    sweep(qacc[g], lambda dc, g=g: wq_sb[g][:, dc, :])
                    rope_store(qacc[g], qT_sb[g][:, sslice], sslice)

                if si < NSTRIPS - 1:
                    # k and v first: their 1MB weights land well before
                    # the 4MB wq, so the first sweep starts as early as
                    # possible during the DMA-paced startup.
                    sweep_k()
                    vstg = sweep_v()
                    sweep_q(0)
                    transposes(vstg)
                    for g in range(1, HQ):
                        sweep_q(g)
                        if g in (1, 2):
                            # prefetch the head of the next strip
                            j0 = 2 * (g - 1)
                            xts[(si + 1, j0)] = load_xt(si + 1, j0)
                            xts[(si + 1, j0 + 1)] = load_xt(si + 1, j0 + 1)
                else:
                    # last strip: attention waits on the final write of
                    # each persistent tile (whole-tile tracking), so
                    # order the sweeps such that every DVE chain (ropes,
                    # vstg) completes while a later sweep still streams.
                    sweep_q(0)
                    sweep_q(1)
                    sweep_q(2)
                    sweep_k()
                    sweep_q(3)
                    vstg = sweep_v()
                    transposes(vstg)
                if si == 0:
                    # wo prefetch rides behind everything strip-0 needs;
                    # it's only consumed from ~185us (first o_proj).
                    nc.sync.dma_start(
                        wo_sb[:], wo_d.rearrange("(c p) m -> p c m", p=128))

        # -------- phase 2+3: attention with interleaved o_proj --------
        with tc.tile_pool(name="pt", bufs=6) as pt_pool, \
             tc.tile_pool(name="a2tmp", bufs=2) as a2tmp, \
             tc.tile_pool(name="sumac", bufs=2) as sum_pool, \
             tc.tile_pool(name="osb", bufs=2) as osb_pool, \
             tc.tile_pool(name="st_ps", bufs=3, space="PSUM") as st_ps, \
             tc.tile_pool(name="oacc_ps", bufs=2, space="PSUM") as oacc_ps, \
             tc.tile_pool(name="opj_ps", bufs=3, space="PSUM") as opj_ps:

            # ---- o_proj emission machinery (software interleave) ----
            # One "chunk" = 4 accumulating MMs (heads) into one [128,512]
            # PSUM tile + a PSUM->SBUF cast copy; half-strips of
            # [128, 2048] DMA out as soon as their 4 chunks land.
            osb_cur = {}        # (si, half) -> osb tile

            def emit_oproj_chunk(si, mi, on_scalar):
                half = mi // 4
                key = (si, half)
                if key not in osb_cur:
                    osb_cur[key] = osb_pool.tile([128, D // 2], F16,
                                                 tag="osbh",
                                                 name=f"osb{si}_{half}")
                osb = osb_cur[key]
                op = opj_ps.tile([128, 512], F32, tag="opj")
                for h in range(HQ):
                    nc.tensor.matmul(
                        op[:],
                        outhT_sb[h][:, si * 128:(si + 1) * 128],
                        wo_sb[:, h, mi * 512:(mi + 1) * 512],
                        start=(h == 0), stop=(h == HQ - 1))
                mo = (mi % 4) * 512
                if on_scalar:
                    nc.scalar.copy(osb[:, mo:mo + 512], op[:])
                else:
                    nc.vector.tensor_copy(osb[:, mo:mo + 512], op[:])
                if si == NPCH - 1 and half == 1:
                    # last half-strip of the kernel: DMA per chunk so the
                    # final transfer is 128KB, not 512KB (shorter drain)
                    nc.sync.dma_start(
                        out_d[si * 128:(si + 1) * 128,
                              2048 + mo:2048 + mo + 512],
                        osb[:, mo:mo + 512])
                    if mi == 7:
                        del osb_cur[key]
                elif mi % 4 == 3:
                    nc.sync.dma_start(
                        out_d[si * 128:(si + 1) * 128,
                              half * 2048:(half + 1) * 2048],
                        osb[:])
                    del osb_cur[key]

            def emit_block(qsrc):
                # solid o_proj block for q-tile qsrc's 4 row-strips.
                # Copies alternate ScalarE/DVE -- no exps compete inside
                # a block and both engines have slack.
                flip = False
                for si in range(qsrc * RD, (qsrc + 1) * RD):
                    for mi in range(D // 512):
                        emit_oproj_chunk(si, mi, flip)
                        flip = not flip

            # ---- attention ----
            # Softmax denominators: instead of a ones-matmul per chunk
            # (160 extra 512-col PE streams ~= 31us), a DVE add folds
            # each exp tile into a per-head [128,512] fp16 partial-sum;
            # one ones-matmul per (qi,h) then reduces the partitions.
            # The PV matmul for chunk c-1 is emitted AFTER the score
            # matmul of chunk c (software pipelining) so its exp/mask
            # dependency is satisfied long before PE reaches it.
            pending = []        # o_proj chunks ready to interleave
            flip = [False]

            def inject_oproj(n=1):
                for _ in range(n):
                    if pending:
                        si, mi = pending.pop(0)
                        emit_oproj_chunk(si, mi, flip[0])
                        flip[0] = not flip[0]

            for qi in range(NQT):
                q0 = qi * QTILE
                npi = RD * (qi + 1)  # causal: key chunks [0, npi)
                # spread the o_proj backlog uniformly over this q-tile's
                # attention chunks (early q-tiles are ACT-bound, so the
                # injected PE work rides free; late q-tiles have more
                # chunks than backlog and stay PE-bound)
                len0, total, done, injected = len(pending), HQ * npi, 0, 0
                for h in range(HQ):
                    oacc = oacc_ps.tile([128, QTILE], F32, tag="oacc")
                    sumac = sum_pool.tile([128, QTILE], F16, tag="sumac")
                    prev = None
                    pts = {}
                    for c in range(npi):
                        r = c - RD * qi          # >=0: diagonal chunk
                        off = 128 * r if r >= 0 else 0
                        diag = r >= 0
                        st = st_ps.tile([128, QTILE], F32, tag="st")
                        nc.tensor.matmul(
                            st[:, off:QTILE],
                            kT_sb[:, c * 128:(c + 1) * 128],
                            qT_sb[h][:, q0 + off:q0 + QTILE],
                            start=True, stop=True)
                        pt = pt_pool.tile([128, QTILE], F16, tag="pt",
                                          name=f"pt{c % 6}")
                        nc.scalar.activation(pt[:, off:QTILE],
                                             st[:, off:QTILE], AF.Exp,
                                             bias=ebias[:])
                        if diag:
                            # zero the in-chunk causal triangle
                            nc.vector.tensor_mul(
                                pt[:, off:off + 128],
                                pt[:, off:off + 128], mask_sb[:])
                        if c == 0:
                            nc.vector.tensor_copy(sumac[:], pt[:])
                        else:
                            nc.vector.tensor_add(sumac[:, off:QTILE],
                                                 sumac[:, off:QTILE],
                                                 pt[:, off:QTILE])
                        if prev is not None:
                            pc, poff = prev
                            nc.tensor.matmul(
                                oacc[:, poff:QTILE],
                                vnat_sb[pc // NSUB][:, pc % NSUB, :],
                                pts[pc % 6][:, poff:QTILE],
                                start=(pc == 0), stop=False,
                                skip_group_check=True)
                            done += 1
                            target = done * len0 // total
                            if injected < target:
                                inject_oproj(target - injected)
                                injected = target
                        pts[c % 6] = pt
                        prev = (c, off)
                    pc, poff = prev
                    nc.tensor.matmul(
                        oacc[:, poff:QTILE],
                        vnat_sb[pc // NSUB][:, pc % NSUB, :],
                        pts[pc % 6][:, poff:QTILE],
                        start=(pc == 0), stop=True,
                        skip_group_check=True)

                    # denominator: one partition-reduce matmul over the
                    # accumulated exp sums, then reciprocal -> broadcast
                    # over the 128 hd partitions -> scale oacc into outhT.
                    # The [1,512] sum row borrows an opj-pool buffer (row
                    # 0) rather than holding a dedicated PSUM bank -- the
                    # freed bank pays for st bufs=3 (deeper score slack).
                    sacc = opj_ps.tile([128, QTILE], F32, tag="opj",
                                       name=f"sacc{qi}_{h}")
                    nc.tensor.matmul(sacc[0:1, :], ones_col[:], sumac[:],
                                     start=True, stop=True)
                    srow = a2tmp.tile([1, QTILE], F32, tag="srow")
                    nc.vector.tensor_copy(srow[:], sacc[0:1, :])
                    rrow = a2tmp.tile([1, QTILE], F32, tag="rrow")
                    nc.vector.reciprocal_approx_fast(rrow[:], srow[:])
                    rbr = a2tmp.tile([128, QTILE], F32, tag="rbr")
                    nc.gpsimd.partition_broadcast(rbr[:], rrow[:],
                                                  channels=128)
                    nc.vector.tensor_mul(outhT_sb[h][:, q0:q0 + QTILE],
                                         oacc[:], rbr[:])

                # this q-tile's o_proj rows are now available; leftovers
                # from earlier q-tiles stay queued -- draining them here
                # would stall on the norm chain just emitted, while the
                # next q-tile's injection ratio absorbs them for free
                for si in range(qi * RD, (qi + 1) * RD):
                    for mi in range(D // 512):
                        pending.append((si, mi))

            # last q-tile's o_proj: nothing left to overlap with; its
            # h3 matmuls wait on the final norm chain (partially hidden
            # by the PE wait-queue running h0-h2 columns ahead).
            inject_oproj(len(pending))
        outh_pool_cm.__exit__(None, None, None)
        wo_pool_cm.__exit__(None, None, None)


_NC_CACHE = None
LAST_RESULT = None
RUN_KWARGS = {}


def _get_nc():
    global _NC_CACHE
    if _NC_CACHE is None:
        _NC_CACHE = build()
    return _NC_CACHE


def kernel(x, wq, wk, wv, wo):
    global LAST_RESULT
    x = np.asarray(x, dtype=np.float32).reshape(S, D)
    xt = np.ascontiguousarray(x.T.astype(np.float16))
    wq = (np.asarray(wq, dtype=np.float32)
          * np.float32(1.0 / np.sqrt(HD))).astype(np.float16)
    wk = np.asarray(wk, dtype=np.float32).astype(np.float16)
    wv = np.asarray(wv, dtype=np.float32).astype(np.float16)
    wo = np.asarray(wo, dtype=np.float32).astype(np.float16)

    in_maps = []
    for c in range(NCORES):
        in_maps.append({
            "xt": xt,
            "wq": np.ascontiguousarray(wq[:, c * NQ:(c + 1) * NQ]),
            "wk": np.ascontiguousarray(wk[:, c * NKV:(c + 1) * NKV]),
            "wv": np.ascontiguousarray(wv[:, c * NKV:(c + 1) * NKV]),
            "wo": np.ascontiguousarray(wo[c * NQ:(c + 1) * NQ, :]),
        })

    nc = _get_nc()
    res = bass_utils.run_bass_kernel_spmd(nc, in_maps,
                                          core_ids=list(range(NCORES)),
                                          **RUN_KWARGS)
    LAST_RESULT = res
    acc = np.zeros((S, D), dtype=np.float64)
    for c in range(NCORES):
        acc += res.results[c]["out"].astype(np.float64)
    return acc.astype(np.float32).reshape(1, S, D)


# revision 48
# speedup vs baseline: 1.0073x; 1.0073x over previous
"""Trainium2 Bass kernel for GQA attention with RoPE (tensor-parallel over heads).

Reference computation (per problem spec):
  x:[1,2048,4096], wq:[4096,4096], wk/wv:[4096,1024], wo:[4096,4096], f32
  q/k/v proj -> RoPE(q,k) -> causal GQA softmax attention -> o_proj

Sharding: 8 cores, tensor-parallel over heads. Core c gets 4 query heads
(wq cols [c*512:(c+1)*512]) and 1 KV head (wk/wv cols [c*128:(c+1)*128]),
plus wo rows [c*512:(c+1)*512]. Each core computes a full [2048,4096]
partial o_proj output; the host sums the 8 partials (the all-reduce).
The host dispatch layer hands the device x pre-transposed ([D,S]).

Matmul operands are fp16 (fast-weight-load hides LDWEIGHTS under the
512-col stream; steady cadence = 216 ns/MM = N/2.4GHz + dispatch).
All accumulation is fp32 in PSUM; softmax statistics fp32.

Trace-driven optimizations over the 477us baseline (now ~388us):
  - softmax denominators: the per-chunk ones-matmul (160 extra 512-col
    PE streams ~= 31us) is replaced by a DVE add folding each exp tile
    into a per-head [128,512] fp16 partial-sum, reduced by ONE
    ones-matmul per (qi,h). Frees a PSUM bank (st*2 + oacc*2 + sacc +
    opj*3 = 8).
  - software pipelining: the PV matmul of chunk c-1 is emitted after
    the score matmul of chunk c, so its exp/mask dependency is long
    satisfied when PE reaches it (the PE 4-deep wait queue covers the
    rest).
  - attention runs at single-chunk granularity ([128,512] score tiles,
    one exp ACT per chunk); oacc double-buffering kills the head-
    boundary stall where PV-start(h+1) waited on the norm mul(h).
  - diagonal score/PV matmuls narrowed to the causally-valid column
    range; the in-chunk triangle is one [128,128] mask multiply.
  - o_proj chunks are interleaved into later q-tiles' chunk loops at a
    rate proportional to the backlog (early q-tiles are ACT-bound, so
    injected PE work rides free); leftovers drain at q-tile ends.
  - dependency granularity: wq/qT/outhT are per-head tiles and vnat
    per-strip (tile-granular tracking -- a consumer only waits on the
    producer it actually reads, not on the tile's latest writer).
  - warm-up: dummy gpsimd partition_broadcast loads the Q7 library and
    a dummy exp loads the ScalarE activation table during phase-1
    startup (each otherwise costs 1.3-7us at the phase boundary).
  - startup DMA schedule in consumption order (k/v sweeps first: their
    1MB weights land before the 4MB wq; consts later); xt prefetched
    one strip ahead; output DMAs per [128,2048] half-strip, per-chunk
    for the final strip so the last transfer is 128KB.
"""
import numpy as np

import concourse.bass as bass
import concourse.bacc as bacc
import concourse.tile as tile
import concourse.mybir as mybir
from concourse import bass_utils

F32 = mybir.dt.float32
F16 = mybir.dt.float16
AF = mybir.ActivationFunctionType

# model dims (hardcoded per problem spec nn_Attention_52020643889298)
S = 2048
D = 4096
H = 32
KV = 8
HD = 128
THETA = 10000.0
NCORES = 8
HQ = H // NCORES            # 4 query heads per core
NQ = HQ * HD                # 512 wq cols per core
NKV = (KV // NCORES) * HD   # 128 wk/wv cols per core

# tiling
SSTRIP = 512                # phase-1 s-strip
NSTRIPS = S // SSTRIP       # 4
NSUB = SSTRIP // 128        # 4
DCH = D // 128              # 32 contraction chunks
QTILE = 512                 # attention q-tile
NQT = S // QTILE            # 4
RD = QTILE // 128           # 4 key chunks per q-tile on the diagonal
NPCH = S // 128             # 16 key chunks

EXP_BIAS = -10.0            # exp(s-10): keeps exp in fp16 range; cancels
                            # in the softmax normalization


def _rope_tables():
    inv = 1.0 / (THETA ** (np.arange(0, HD, 2, dtype=np.float64) / HD))
    pos = np.arange(S, dtype=np.float64)
    freqs = pos[:, None] * inv[None, :]          # [S, 64]
    emb = np.concatenate([freqs, freqs], axis=1)  # [S, HD]
    cosT = np.cos(emb).T.astype(np.float16).copy()  # [HD, S]
    sinT = np.sin(emb).T.astype(np.float16).copy()
    return cosT, sinT


def _mask_tri():
    # mask[p, j] = 1 iff j >= p: the in-chunk causal triangle. Every
    # diagonal [128-key x 128-query] block uses this same mask after the
    # score tile is narrowed to its first causally-live 128 columns.
    p = np.arange(128)[:, None]
    j = np.arange(128)[None, :]
    return (j >= p).astype(np.float16)


def build():
    nc = bacc.Bacc("TRN2", target_bir_lowering=False, debug=False,
                   enable_asserts=False, num_devices=NCORES)
    xt_d = nc.dram_tensor("xt", [D, S], F16, kind="ExternalInput").ap()
    wq_d = nc.dram_tensor("wq", [D, NQ], F16, kind="ExternalInput").ap()
    wk_d = nc.dram_tensor("wk", [D, NKV], F16, kind="ExternalInput").ap()
    wv_d = nc.dram_tensor("wv", [D, NKV], F16, kind="ExternalInput").ap()
    wo_d = nc.dram_tensor("wo", [NQ, D], F16, kind="ExternalInput").ap()
    out_d = nc.dram_tensor("out", [S, D], F16, kind="ExternalOutput").ap()

    cosT, sinT = _rope_tables()
    ident_d = nc.inline_tensor(np.eye(128, dtype=np.float16), "ident").ap()
    cos_d = nc.inline_tensor(cosT, "cosT").ap()
    sin_d = nc.inline_tensor(sinT, "sinT").ap()
    mask_d = nc.inline_tensor(_mask_tri(), "masktri").ap()
    ones_d = nc.inline_tensor(np.ones((128, 1), np.float16), "onescol").ap()
    ebias_d = nc.inline_tensor(
        np.full((128, 1), EXP_BIAS, np.float32), "ebias").ap()

    with tile.TileContext(nc) as tc:
        _body(nc, tc, xt_d, wq_d, wk_d, wv_d, wo_d, out_d,
              ident_d, cos_d, sin_d, mask_d, ones_d, ebias_d)
    nc.compile()
    return nc


def _body(nc, tc, xt_d, wq_d, wk_d, wv_d, wo_d, out_d,
          ident_d, cos_d, sin_d, mask_d, ones_d, ebias_d):
    wqr = wq_d.rearrange("(c p) n -> p c n", p=128)
    wkr = wk_d.rearrange("(c p) n -> p c n", p=128)
    wvr = wv_d.rearrange("(c p) n -> p c n", p=128)
    xtr = xt_d.rearrange("(c p) s -> p c s", p=128)  # [128, DCH, S]

    with tc.tile_pool(name="const", bufs=1) as const_pool, \
         tc.tile_pool(name="persist", bufs=1) as persist:

        # persistent activations. qT is one tile per head and vnat one
        # tile per strip: dependency tracking is tile-granular, so the
        # first q-tile's score/PV matmuls only wait on the strips they
        # actually read, not on the last strip's rope/transpose chains.
        qT_sb = [persist.tile([128, S], F16, name=f"qT{g}")
                 for g in range(HQ)]               # [hd, s] x HQ
        kT_sb = persist.tile([128, S], F16)        # [hd, s]
        vnat_sb = [persist.tile([128, NSUB, HD], F16, name=f"vnat{si}")
                   for si in range(NSTRIPS)]       # [s%128, sub, hd]

        # ---------------- phase 1: QKV projection + RoPE ----------------
        wo_pool_cm = tc.tile_pool(name="wo2", bufs=1)
        outh_pool_cm = tc.tile_pool(name="outh", bufs=1)
        wo_pool = wo_pool_cm.__enter__()
        outh_pool = outh_pool_cm.__enter__()
        wo_sb = wo_pool.tile([128, HQ, D], F16)
        # one tile per head: tile-granular dependency tracking means an
        # o_proj matmul only waits on ITS head's latest norm mul, not on
        # whatever head was normalized most recently.
        outhT_sb = [outh_pool.tile([128, S], F16, name=f"outh{h}")
                    for h in range(HQ)]  # [hd, s] x HQ
        with tc.tile_pool(name="rope_c", bufs=1) as rope_c, \
             tc.tile_pool(name="w1", bufs=1) as w1, \
             tc.tile_pool(name="xt", bufs=12) as xt_pool, \
             tc.tile_pool(name="p1tmp", bufs=2) as p1tmp, \
             tc.tile_pool(name="tp_ps", bufs=2, space="PSUM") as tp_ps, \
             tc.tile_pool(name="acc_ps", bufs=1, space="PSUM") as acc_ps:

            # wq as one tile per head block: a q-sweep then only waits on
            # its own 1MB DMA, not on all 4MB of wq (tile-granular deps)
            wq_sb = [w1.tile([128, DCH, 128], F16, name=f"wq{g}")
                     for g in range(HQ)]
            wk_sb = w1.tile([128, DCH, NKV], F16)
            wv_sb = w1.tile([128, DCH, NKV], F16)

            XG = 4  # d-chunks per xt DMA (512KB per transfer)

            def load_xt(si, j):
                t = xt_pool.tile([128, XG, SSTRIP], F16, tag="xt",
                                 name=f"xt{si}_{j}")
                nc.sync.dma_start(
                    t[:], xtr[:, j * XG:(j + 1) * XG,
                              si * SSTRIP:(si + 1) * SSTRIP])
                return t

            # Startup DMA schedule, in consumption order: the k-sweep
            # (first sweep, DMA-paced) reads xt chunk groups in j order
            # plus the 1MB wk; wv before the xt tail so the v-sweep never
            # waits; the 4MB wq halves and DVE-only consts ride behind.
            xts = {}
            xts[(0, 0)] = load_xt(0, 0)
            nc.sync.dma_start(wk_sb[:], wkr[:])
            xts[(0, 1)] = load_xt(0, 1)
            xts[(0, 2)] = load_xt(0, 2)
            xts[(0, 3)] = load_xt(0, 3)
            nc.sync.dma_start(wv_sb[:], wvr[:])
            xts[(0, 4)] = load_xt(0, 4)
            xts[(0, 5)] = load_xt(0, 5)
            xts[(0, 6)] = load_xt(0, 6)
            xts[(0, 7)] = load_xt(0, 7)

            ebias = const_pool.tile([128, 1], F32)
            nc.sync.dma_start(ebias[:], ebias_d[:])
            ones_col = const_pool.tile([128, 1], F16)
            nc.sync.dma_start(ones_col[:], ones_d[:])
            ident = const_pool.tile([128, 128], F16)
            nc.sync.dma_start(ident[:], ident_d[:])

            # Preload the gpsimd library (partition_broadcast lives in a
            # Q7 library that otherwise lazy-loads at first use -- ~7us of
            # dead time right at the attention phase boundary). This dummy
            # broadcast hides the load under the strip-0 sweeps. Same idea
            # for ScalarE's Exp activation table (~1.3us ACT_TABLE_LOAD
            # that would otherwise land on the first real exp).
            dummy_bc = const_pool.tile([128, 1], F32)
            nc.gpsimd.partition_broadcast(dummy_bc[:], ebias[0:1, :],
                                          channels=128)
            dummy_exp = const_pool.tile([128, 1], F16)
            nc.scalar.activation(dummy_exp[:], ebias[:], AF.Exp,
                                 bias=ebias[:])

            for g in range(HQ):
                nc.sync.dma_start(wq_sb[g][:],
                                  wqr[:, :, g * 128:(g + 1) * 128])
            cos_sb = rope_c.tile([128, S], F16)
            nc.sync.dma_start(cos_sb[:], cos_d[:])
            sin_sb = rope_c.tile([128, S], F16)
            nc.sync.dma_start(sin_sb[:], sin_d[:])
            mask_sb = const_pool.tile([128, 128], F16)
            nc.sync.dma_start(mask_sb[:], mask_d[:])

            def rope_store(src_ps, dst_ap, sslice):
                # dst = src*cos + rot(src)*sin, rot = [-src[64:], src[:64]].
                # SBUF+SBUF DVE operands must share their base partition, so
                # materialize the half-rotated src from PSUM first, then all
                # remaining ops are partition-aligned fp16 SBUF math.
                qrot = p1tmp.tile([128, SSTRIP], F16, tag="rope_qr",
                                  name="rope_qr")
                nc.vector.tensor_copy(qrot[0:64, :], src_ps[64:128, :])
                nc.vector.tensor_copy(qrot[64:128, :], src_ps[0:64, :])
                qcos = p1tmp.tile([128, SSTRIP], F16, tag="rope_qc",
                                  name="rope_qc")
                nc.vector.tensor_mul(qcos[:], src_ps[:], cos_sb[:, sslice])
                nc.vector.tensor_mul(qrot[:], qrot[:], sin_sb[:, sslice])
                nc.vector.tensor_sub(dst_ap[0:64, :], qcos[0:64, :],
                                     qrot[0:64, :])
                nc.vector.tensor_add(dst_ap[64:128, :], qcos[64:128, :],
                                     qrot[64:128, :])

            for si in range(NSTRIPS):
                s0 = si * SSTRIP
                sslice = slice(s0, s0 + SSTRIP)
                if si > 0:
                    # rest of this strip's x columns (j0-j3 were
                    # prefetched during the previous strip)
                    for j in range(4, DCH // XG):
                        xts[(si, j)] = load_xt(si, j)

                kacc = acc_ps.tile([128, SSTRIP], F32, tag="kacc")
                vacc = acc_ps.tile([128, SSTRIP], F32, tag="vacc")
                qacc = [acc_ps.tile([128, SSTRIP], F32, tag=f"qacc{g}",
                                    name=f"qacc{g}")
                        for g in range(HQ)]

                xtiles = [xts.pop((si, j)) for j in range(DCH // XG)]

                def sweep(acc, wsl):
                    for j in range(DCH // XG):
                        for jj in range(XG):
                            dc = j * XG + jj
                            nc.tensor.matmul(acc[:], wsl(dc),
                                             xtiles[j][:, jj, :],
                                             start=(dc == 0),
                                             stop=(dc == DCH - 1))

                def sweep_k():
                    sweep(kacc, lambda dc: wk_sb[:, dc, :])
                    rope_store(kacc, kT_sb[:, sslice], sslice)

                def sweep_v():
                    # vstg (DVE) is emitted right after the sweep; the PE
                    # transposes are emitted later (after the next sweep)
                    # so they never queue behind a DVE chain in flight.
                    sweep(vacc, lambda dc: wv_sb[:, dc, :])
                    vstg = p1tmp.tile([128, SSTRIP], F16, tag="vstg")
                    nc.vector.tensor_copy(vstg[:], vacc[:])
                    return vstg

                def transposes(vstg):
                    # all 4 transposes land in one PSUM tile, drained by a
                    # single copy (vnat's strip slice is contiguous) -- no
                    # PE<->DVE ping-pong at strip boundaries
                    tp = tp_ps.tile([128, SSTRIP], F16, tag="tp")
                    for ss in range(NSUB):
                        nc.tensor.transpose(
                            tp[:, ss * 128:(ss + 1) * 128],
                            vstg[:, ss * 128:(ss + 1) * 128], ident[:])
                    nc.vector.tensor_copy(vnat_sb[si][:], tp[:])

                def sweep_q(g):
                    sweep(qacc[g], lambda dc, g=g: wq_sb[g][:, dc, :])
                    rope_store(qacc[g], qT_sb[g][:, sslice], sslice)

                if si < NSTRIPS - 1:
                    # k and v first: their 1MB weights land well before
                    # the 4MB wq, so the first sweep starts as early as
                    # possible during the DMA-paced startup.
                    sweep_k()
                    vstg = sweep_v()
                    sweep_q(0)
                    transposes(vstg)
                    for g in range(1, HQ):
                        sweep_q(g)
                        if g in (1, 2):
                            # prefetch the head of the next strip
                            j0 = 2 * (g - 1)
                            xts[(si + 1, j0)] = load_xt(si + 1, j0)
                            xts[(si + 1, j0 + 1)] = load_xt(si + 1, j0 + 1)
                else:
                    # last strip: attention waits on the final write of
                    # each persistent tile (whole-tile tracking), so
                    # order the sweeps such that every DVE chain (ropes,
                    # vstg) completes while a later sweep still streams.
                    sweep_q(0)
                    sweep_q(1)
                    sweep_q(2)
                    sweep_k()
                    sweep_q(3)
                    vstg = sweep_v()
                    transposes(vstg)
                if si == 0:
                    # wo prefetch rides behind everything strip-0 needs;
                    # it's only consumed from ~185us (first o_proj).
                    nc.sync.dma_start(
                        wo_sb[:], wo_d.rearrange("(c p) m -> p c m", p=128))

        # -------- phase 2+3: attention with interleaved o_proj --------
        with tc.tile_pool(name="pt", bufs=6) as pt_pool, \
             tc.tile_pool(name="a2tmp", bufs=2) as a2tmp, \
             tc.tile_pool(name="sumac", bufs=2) as sum_pool, \
             tc.tile_pool(name="osb", bufs=2) as osb_pool, \
             tc.tile_pool(name="st_ps", bufs=3, space="PSUM") as st_ps, \
             tc.tile_pool(name="oacc_ps", bufs=2, space="PSUM") as oacc_ps, \
             tc.tile_pool(name="opj_ps", bufs=3, space="PSUM") as opj_ps:

            # ---- o_proj emission machinery (software interleave) ----
            # One "chunk" = 4 accumulating MMs (heads) into one [128,512]
            # PSUM tile + a PSUM->SBUF cast copy; half-strips of
            # [128, 2048] DMA out as soon as their 4 chunks land.
            osb_cur = {}        # (si, half) -> osb tile

            def emit_oproj_chunk(si, mi, on_scalar):
                half = mi // 4
                key = (si, half)
                if key not in osb_cur:
                    osb_cur[key] = osb_pool.tile([128, D // 2], F16,
                                                 tag="osbh",
                                                 name=f"osb{si}_{half}")
                osb = osb_cur[key]
                op = opj_ps.tile([128, 512], F32, tag="opj")
                for h in range(HQ):
                    nc.tensor.matmul(
                        op[:],
                        outhT_sb[h][:, si * 128:(si + 1) * 128],
                        wo_sb[:, h, mi * 512:(mi + 1) * 512],
                        start=(h == 0), stop=(h == HQ - 1))
                mo = (mi % 4) * 512
                if on_scalar:
                    nc.scalar.copy(osb[:, mo:mo + 512], op[:])
                else:
                    nc.vector.tensor_copy(osb[:, mo:mo + 512], op[:])
                if si == NPCH - 1 and half == 1:
                    # last half-strip of the kernel: DMA per chunk so the
                    # final transfer is 128KB, not 512KB (shorter drain)
                    nc.sync.dma_start(
                        out_d[si * 128:(si + 1) * 128,
                              2048 + mo:2048 + mo + 512],
                        osb[:, mo:mo + 512])
                    if mi == 7:
                        del osb_cur[key]
                elif mi % 4 == 3:
                    nc.sync.dma_start(
                        out_d[si * 128:(si + 1) * 128,
                              half * 2048:(half + 1) * 2048],
                        osb[:])
                    del osb_cur[key]

            def emit_block(qsrc):
                # solid o_proj block for q-tile qsrc's 4 row-strips.
                # Copies alternate ScalarE/DVE -- no exps compete inside
                # a block and both engines have slack.
                flip = False
                for si in range(qsrc * RD, (qsrc + 1) * RD):
                    for mi in range(D // 512):
                        emit_oproj_chunk(si, mi, flip)
                        flip = not flip

            # ---- attention ----
            # Softmax denominators: instead of a ones-matmul per chunk
            # (160 extra 512-col PE streams ~= 31us), a DVE add folds
            # each exp tile into a per-head [128,512] fp16 partial-sum;
            # one ones-matmul per (qi,h) then reduces the partitions.
            # The PV matmul for chunk c-1 is emitted AFTER the score
            # matmul of chunk c (software pipelining) so its exp/mask
            # dependency is satisfied long before PE reaches it.
            pending = []        # o_proj chunks ready to interleave
            flip = [False]

            def inject_oproj(n=1):
                for _ in range(n):
                    if pending:
                        si, mi = pending.pop(0)
                        emit_oproj_chunk(si, mi, flip[0])
                        flip[0] = not flip[0]

            for qi in range(NQT):
                q0 = qi * QTILE
                npi = RD * (qi + 1)  # causal: key chunks [0, npi)
                # spread the o_proj backlog uniformly over this q-tile's
                # attention chunks (early q-tiles are ACT-bound, so the
                # injected PE work rides free; late q-tiles have more
                # chunks than backlog and stay PE-bound)
                len0, total, done, injected = len(pending), HQ * npi, 0, 0
                for h in range(HQ):
                    oacc = oacc_ps.tile([128, QTILE], F32, tag="oacc")
                    sumac = sum_pool.tile([128, QTILE], F16, tag="sumac")
                    prev = None
                    pts = {}
                    for c in range(npi):
                        r = c - RD * qi          # >=0: diagonal chunk
                        off = 128 * r if r >= 0 else 0
                        diag = r >= 0
                        st = st_ps.tile([128, QTILE], F32, tag="st")
                        nc.tensor.matmul(
                            st[:, off:QTILE],
                            kT_sb[:, c * 128:(c + 1) * 128],
                            qT_sb[h][:, q0 + off:q0 + QTILE],
                            start=True, stop=True)
                        pt = pt_pool.tile([128, QTILE], F16, tag="pt",
                                          name=f"pt{c % 6}")
                        nc.scalar.activation(pt[:, off:QTILE],
                                             st[:, off:QTILE], AF.Exp,
                                             bias=ebias[:])
                        if diag:
                            # zero the in-chunk causal triangle
                            nc.vector.tensor_mul(
                                pt[:, off:off + 128],
                                pt[:, off:off + 128], mask_sb[:])
                        if c == 0:
                            nc.vector.tensor_copy(sumac[:], pt[:])
                        else:
                            nc.vector.tensor_add(sumac[:, off:QTILE],
                                                 sumac[:, off:QTILE],
                                                 pt[:, off:QTILE])
                        if prev is not None:
                            pc, poff = prev
                            nc.tensor.matmul(
                                oacc[:, poff:QTILE],
                                vnat_sb[pc // NSUB][:, pc % NSUB, :],
                                pts[pc % 6][:, poff:QTILE],
                                start=(pc == 0), stop=False,
                                skip_group_check=True)
                            done += 1
                            target = done * len0 // total
                            if injected < target:
                                inject_oproj(target - injected)
                                injected = target
                        pts[c % 6] = pt
                        prev = (c, off)
                    pc, poff = prev
                    nc.tensor.matmul(
                        oacc[:, poff:QTILE],
                        vnat_sb[pc // NSUB][:, pc % NSUB, :],
                        pts[pc % 6][:, poff:QTILE],
                        start=(pc == 0), stop=True,
                        skip_group_check=True)

                    # denominator: one partition-reduce matmul over the
                    # accumulated exp sums, then reciprocal -> broadcast
                    # over the 128 hd partitions -> scale oacc into outhT.
                    # The [1,512] sum row borrows an opj-pool buffer (row
                    # 0) rather than holding a dedicated PSUM bank -- the
                    # freed bank pays for st bufs=3 (deeper score slack).
                    sacc = opj_ps.tile([128, QTILE], F32, tag="opj",
                                       name=f"sacc{qi}_{h}")
                    nc.tensor.matmul(sacc[0:1, :], ones_col[:], sumac[:],
                                     start=True, stop=True)
                    srow = a2tmp.tile([1, QTILE], F32, tag="srow")
                    nc.vector.tensor_copy(srow[:], sacc[0:1, :])
                    rrow = a2tmp.tile([1, QTILE], F32, tag="rrow")
                    nc.vector.reciprocal_approx_fast(rrow[:], srow[:])
                    rbr = a2tmp.tile([128, QTILE], F32, tag="rbr")
                    nc.gpsimd.partition_broadcast(rbr[:], rrow[:],
                                                  channels=128)
                    nc.vector.tensor_mul(outhT_sb[h][:, q0:q0 + QTILE],
                                         oacc[:], rbr[:])

                # this q-tile's o_proj rows are now available; leftovers
                # from earlier q-tiles stay queued -- draining them here
                # would stall on the norm chain just emitted, while the
                # next q-tile's injection ratio absorbs them for free
                for si in range(qi * RD, (qi + 1) * RD):
                    for mi in range(D // 512):
                        pending.append((si, mi))

            # last q-tile's o_proj: nothing left to overlap with; its
            # h3 matmuls wait on the final norm chain (partially hidden
            # by the PE wait-queue running h0-h2 columns ahead).
            inject_oproj(len(pending))
        outh_pool_cm.__exit__(None, None, None)
        wo_pool_cm.__exit__(None, None, None)


_NC_CACHE = None
LAST_RESULT = None
RUN_KWARGS = {}


def _get_nc():
    global _NC_CACHE
    if _NC_CACHE is None:
        _NC_CACHE = build()
    return _NC_CACHE


def kernel(x, wq, wk, wv, wo):
    global LAST_RESULT
    x = np.asarray(x, dtype=np.float32).reshape(S, D)
    xt = np.ascontiguousarray(x.T.astype(np.float16))
    wq = (np.asarray(wq, dtype=np.float32)
          * np.float32(1.0 / np.sqrt(HD))).astype(np.float16)
    wk = np.asarray(wk, dtype=np.float32).astype(np.float16)
    wv = np.asarray(wv, dtype=np.float32).astype(np.float16)
    wo = np.asarray(wo, dtype=np.float32).astype(np.float16)

    in_maps = []
    for c in range(NCORES):
        in_maps.append({
            "xt": xt,
            "wq": np.ascontiguousarray(wq[:, c * NQ:(c + 1) * NQ]),
            "wk": np.ascontiguousarray(wk[:, c * NKV:(c + 1) * NKV]),
            "wv": np.ascontiguousarray(wv[:, c * NKV:(c + 1) * NKV]),
            "wo": np.ascontiguousarray(wo[c * NQ:(c + 1) * NQ, :]),
        })

    nc = _get_nc()
    res = bass_utils.run_bass_kernel_spmd(nc, in_maps,
                                          core_ids=list(range(NCORES)),
                                          **RUN_KWARGS)
    LAST_RESULT = res
    acc = np.zeros((S, D), dtype=np.float64)
    for c in range(NCORES):
        acc += res.results[c]["out"].astype(np.float64)
    return acc.astype(np.float32).reshape(1, S, D)
